# revision 62
# baseline (speedup 1.0000x reference)
"""Trainium2 Bass kernel for AverageSpanExtractor (segment mean over spans).

Math note: the reference's masked softmax over all-ones logits reduces
exactly to a mean over the span tokens [start, end):
    out[b, n, :] = mean(sequence_tensor[b, start:end, :]).

Strategy (8 cores, batch-parallel - one batch element per core):
  1. Phase 1 builds a DRAM table `cum` (fp16) of block-local inclusive
     prefix sums: cum[1+t] = sum of seq rows [128*(t>>7) .. t].  The
     host ships seq as fp16 (error budget 2e-2; fp16 seq costs ~5e-4)
     and tokens load PACKED: partition p holds tokens 8p..8p+7 of its
     1024-token group, so every load/store descriptor is 4 KB
     contiguous (128/DMA vs 1024 token-major) - HWDGE descriptor rate
     is the phase-1 ceiling otherwise.  The prefix splits into an
     in-place fp16 DVE column chain (7 adds over the 8 packed tokens),
     a host-built block-diagonal strict-triangular matmul M
     (16-partition chunks = 128-token blocks) for the inter-partition
     part, and an fp16 broadcast-add.  The last group's broadcast-add
     and store are split in half to overlap the store with compute.
  2. Span sum = cum[e] - cum[s] + corr; a width<=32 span crosses at
     most one block boundary; corr = Ttab[gb], gb = (qe-qs)*qe,
     q* = (x-1)>>7, Ttab[k] = cum[128k] (row 0 = zeros).
  3. The 2048 cum rows are fetched by four 512-row dma_gather ops on
     four SWDGE queues.  dma_gather lives in the Q7 'mlp' ucode
     library: a dummy 128-row warmup gather right at program start
     absorbs the one-time library load (+drain) that otherwise lands
     between the cum stores and the real gathers.  No other Pool-engine
     library ops exist (M/one-hots/iota work all moved to the host), so
     no mid-program reload.  Desc-gen (~4.5 us Q7 time for the first
     512-row gather, the rest overlap on separate core pairs) starts
     when the last cum store lands.
  4. corr = onehot(gb) @ Ttab on the otherwise-idle PE; one-hots and
     1/width come precomputed from the host (index metadata only).
     Combine = one DVE subtract + batched corr-adds + one batched
     1/w multiply (wrec broadcast along d) per half; output stores
     split across both HWDGE queues.
"""

import numpy as np

B, S, D = 8, 4096, 256
N_SPANS = 1024
P = 128
NBLK = S // P          # 32 blocks of 128 tokens
JG = N_SPANS // P      # 8 spans per partition
MPK = 8                # tokens packed per partition per load group
NG = S // (P * MPK)    # 4 load groups of 1024 tokens
NT = 33                # block-total table rows (incl. zero row)

_cached_nc = None


def build_nc():
    import copy as _copy
    import re as _re

    import concourse.bass as bass
    import concourse.bass_isa as bass_isa
    import concourse.bacc as bacc
    import concourse.mybir as mybir
    from concourse.tile import TileContext
    from concourse.instruction_name_ordered_set import InstructionNameOrderedSet

    f32 = mybir.dt.float32
    f16 = mybir.dt.float16
    i16 = mybir.dt.int16
    Alu = mybir.AluOpType
    Act = mybir.ActivationFunctionType

    nc = bacc.Bacc(
        None, target_bir_lowering=False, debug=False, num_devices=B,
        num_swdge_queues=4,
    )
    seq = nc.declare_dram_parameter("seq16", [S, D], f16, isOutput=False)
    # spans16[q, 32k + c*8 + r] = (k<2 ? end : start) of span
    # 8*(16r+q) + (k%2)*4 + c, replicated across the 8 16-row groups.
    spans16 = nc.declare_dram_parameter("spans16", [P, 128], i16, isOutput=False)
    # host-precomputed metadata: 1/width, corr one-hots, chunk-tri matrix
    wrec = nc.declare_dram_parameter("wrec", [P, JG], f32, isOutput=False)
    ohs = nc.declare_dram_parameter("ohs", [NT, JG * P], f16, isOutput=False)
    m16 = nc.declare_dram_parameter("m16", [P, P], f16, isOutput=False)
    out = nc.declare_dram_parameter("out", [N_SPANS, D], f32, isOutput=True)

    with TileContext(nc) as tc:
        with (
            tc.tile_pool(name="const", bufs=1) as const_pool,
            tc.tile_pool(name="x", bufs=NG) as x_pool,
            tc.tile_pool(name="c", bufs=NG) as c_pool,
            tc.tile_pool(name="ps", bufs=3, space="PSUM") as ps_pool,
            tc.tile_pool(name="cr", bufs=4, space="PSUM") as cr_pool,
            tc.tile_pool(name="misc", bufs=1) as misc_pool,
            tc.tile_pool(name="g", bufs=1) as g_pool,
            tc.tile_pool(name="res", bufs=1) as res_pool,
            tc.tile_pool(name="dram", bufs=1, space="DRAM") as d_pool,
        ):
            # DRAM scratch: block-local prefix rows in fp16; row 0 zeros.
            cum = d_pool.tile([S + 1, D], f16)

            # --- big seq loads, packed [p, m, d]: 4 KB contiguous per
            # partition, 128 descriptors each; alternate SP/ACT queues ---
            bigxs = []
            # high_priority: the scheduler otherwise slots the small
            # metadata loads ahead of these on both HWDGE queues, delaying
            # the first seq descriptors (and everything downstream) ~1.5us
            with tc.high_priority():
                for g in range(NG):
                    t0 = g * MPK * P
                    bigx = x_pool.tile([P, MPK * D], f16, name=f"bigx{g}")
                    eng = nc.sync if g % 2 == 0 else nc.scalar
                    eng.dma_start(
                        out=bigx[:],
                        in_=seq[t0 : t0 + MPK * P, :].rearrange(
                            "(p m) d -> p m d", p=P
                        ),
                    )
                    bigxs.append(bigx)

            # --- small metadata loads ---
            wr = misc_pool.tile([P, JG], f32)
            nc.sync.dma_start(out=wr[:], in_=wrec[:])
            M = const_pool.tile([P, P], f16)
            nc.sync.dma_start(out=M[:], in_=m16[:])
            I16 = misc_pool.tile([P, 128], i16)
            nc.scalar.dma_start(out=I16[:], in_=spans16[:])
            ohS = misc_pool.tile([NT, JG * P], f16)
            nc.scalar.dma_start(out=ohS[:], in_=ohs[:])
            zrow = const_pool.tile([1, D], f16)
            nc.vector.memset(zrow[:], 0.0)
            nc.scalar.dma_start(out=cum[0:1, :], in_=zrow[:])

            # --- gather PREPS: Q7 desc-gen (~4.5 us) reads only I16 and
            # runs on the otherwise-idle Pool engine during phase 1; the
            # cum data read happens at trigger time.  Emitted before the
            # cum stores so no RAW dep lands on the preps.  G is memset
            # first so the scheduling sim (which models prep-written data
            # as ready at prep time) never reads uninitialized memory -
            # the real ordering is enforced by the patched gdma waits. ---
            G = g_pool.tile([P, 16 * D], f16)
            Gv = G[:].rearrange("p (c d) -> p c d", d=D)
            nc.gpsimd.memset(G[:], 0.0)
            preps = []
            for q, k in enumerate((0, 2, 1, 3)):
                dsem = nc.alloc_semaphore(f"gdma{q}")
                preps.append(
                    nc.gpsimd.dma_gather(
                        Gv[:, 4 * k : 4 * (k + 1), :],
                        cum[:],
                        I16[:, 32 * k : 32 * (k + 1)],
                        4 * P,
                        4 * P,
                        D,
                        queue_num=q,
                        prepare_only=True,
                        sem=dsem,
                    )
                )

            # --- phase 1 per group: in-place fp16 column chain
            # (intra-partition prefix over the 8 packed tokens), chunk-tri
            # matmul for the inter-partition part, fp16 broadcast-add,
            # store ---
            bigcs = []
            for g in range(NG):
                t0 = g * MPK * P
                bigx = bigxs[g]
                v = bigx[:].rearrange("p (m d) -> p m d", m=MPK)
                for m in range(1, MPK):
                    nc.vector.tensor_tensor(
                        out=v[:, m, :], in0=v[:, m, :], in1=v[:, m - 1, :],
                        op=Alu.add,
                    )
                part = ps_pool.tile([P, 512], f32)
                nc.tensor.matmul(
                    out=part[:, 0:D], lhsT=M[:], rhs=v[:, MPK - 1, :],
                    start=True, stop=True,
                )
                p16 = misc_pool.tile([P, D], f16, name=f"p16_{g}")
                nc.scalar.activation(out=p16[:], in_=part[:, 0:D], func=Act.Copy)
                bigc = c_pool.tile([P, MPK * D], f16)
                bigcs.append(bigc)
                cv = bigc[:].rearrange("p (m d) -> p m d", m=MPK)
                pb = p16[:].rearrange("p (m d) -> p m d", m=1)
                eng = nc.sync if g % 2 == 0 else nc.scalar
                # split the last group's broadcast-add + store in half so
                # the store overlaps the second half's compute
                halves = (MPK // 2, MPK) if g == NG - 1 else (MPK,)
                mlo = 0
                for mh in halves:
                    nc.vector.tensor_tensor(
                        out=cv[:, mlo:mh, :], in0=v[:, mlo:mh, :],
                        in1=pb.to_broadcast([P, mh - mlo, D]),
                        op=Alu.add,
                    )
                    eng.dma_start(
                        out=cum[1 + t0 : 1 + t0 + MPK * P, :].rearrange(
                            "(p m) d -> p m d", p=P
                        )[:, mlo:mh, :],
                        in_=bigc[:].rearrange("p (m d) -> p m d", m=MPK)[
                            :, mlo:mh, :
                        ],
                    )
                    mlo = mh

            # --- trigger gating: tiny Pool memsets WAR-depend on each
            # store's source region (and zrow), so they sem-wait store
            # completion ahead of the triggers in the Pool FIFO ---
            gates = [nc.gpsimd.memset(zrow[0:1, 0:4], 0.0)]
            for g in range(NG):
                gates.append(nc.gpsimd.memset(bigcs[g][0:1, 0:4], 0.0))
                if g == NG - 1:
                    gates.append(
                        nc.gpsimd.memset(
                            bigcs[g][0:1, (MPK - 1) * D : (MPK - 1) * D + 4],
                            0.0,
                        )
                    )

            # everything from the triggers on is pinned LAST in the
            # schedule (negative high_priority offset = low priority):
            # the scheduler models prep-written G data as ready at prep
            # time and would otherwise hoist tail ops ahead of phase-1
            # work on shared queues, deadlocking against the
            # store->trigger->gather chain.
            tail = tc.high_priority(offset=-1000000)
            tail.__enter__()
            # manual trigger construction: nosync deps must be attached
            # BEFORE add_instruction for Tile to see them, and the public
            # trigger_dma only links the preps - we also need the gates so
            # neither the scheduler nor the sim fires a trigger before the
            # stores complete.
            trigs = []
            for q in range(4):
                pend = nc.gpsimd._pending_untriggered_insts[q]
                deps = InstructionNameOrderedSet()
                for pi in pend:
                    deps.add(pi.ins.name)
                for gt in gates:
                    deps.add(gt.ins.name)
                kcnt = len(pend)
                nc.gpsimd._pending_untriggered_insts[q] = []
                trigger = bass_isa.InstTriggerDma(
                    name=nc.get_next_instruction_name(),
                    ins=[],
                    outs=[],
                    _count=kcnt,
                    _count_reg=None,
                    queue_num=q,
                )
                trigger.add_nosync_dependencies_from(deps)
                trigs.append(nc.gpsimd.add_instruction(trigger))

            # block-total table: strided fetch of cum rows 0,128,...,4096
            Ttab = misc_pool.tile([NT, D], f16)
            nc.scalar.dma_start(out=Ttab[:], in_=cum[0 : NBLK * P + 1 : P, :])

            # corr_j = onehot(gb_j) @ Ttab on the otherwise-idle PE.
            # Full-bank tiles: a PSUM bank written by PE while DVE reads a
            # bank-mate tile is a fatal HW collision.
            corr = []
            for j in range(JG):
                crj = cr_pool.tile([P, 2 * D], f32)
                nc.tensor.matmul(
                    out=crj[:, 0:D], lhsT=ohS[:, j * P : (j + 1) * P],
                    rhs=Ttab[:], start=True, stop=True,
                )
                corr.append(crj)

            outv = out[:].rearrange("(p jj) d -> p jj d", p=P)
            T1 = res_pool.tile([P, 8 * D], f32)
            T1v = T1[:].rearrange("p (c d) -> p c d", d=D)
            R = res_pool.tile([P, 8 * D], f32)
            Rv = R[:].rearrange("p (c d) -> p c d", d=D)
            subs = []
            for h in range(2):
                # WAW guard: a tiny copy into this sub's output region
                # that RAW-depends on the last broadcast-add, so the
                # scheduler cannot place the sub (which it models as
                # ready at prep time) ahead of phase-1 work on DVE.
                nc.vector.tensor_copy(
                    out=T1[0:1, 4 * h * D : 4 * h * D + 2],
                    in_=bigcs[NG - 1][0:1, 0:2],
                )
                # ends half h lives in Gv cols 4h..4h+4, starts in 8+4h..
                subs.append(
                    nc.vector.tensor_tensor(
                        out=T1v[:, 4 * h : 4 * h + 4, :],
                        in0=Gv[:, 4 * h : 4 * h + 4, :],
                        in1=Gv[:, 8 + 4 * h : 8 + 4 * h + 4, :],
                        op=Alu.subtract,
                    )
                )
                for c in range(4):
                    j = 4 * h + c
                    nc.vector.tensor_tensor(
                        out=T1v[:, j, :], in0=T1v[:, j, :], in1=corr[j][:, 0:D],
                        op=Alu.add,
                    )
                    nc.scalar.activation(
                        out=Rv[:, j, :], in_=T1v[:, j, :], func=Act.Copy,
                        scale=wr[:, j : j + 1],
                    )
                eng = nc.sync if h == 0 else nc.scalar
                eng.dma_start(
                    out=outv[:, 4 * h : 4 * h + 4, :],
                    in_=Rv[:, 4 * h : 4 * h + 4, :],
                )
            tail.__exit__(None, None, None)

    # --- post-pass surgery (Tile's managed path does not support
    # DRAM-source preps): every WAIT referencing a prep's Tile-assigned
    # DMASW lane sem is remapped to the prep's descriptor-encoded gdma
    # sem (a lane sem may only be updated by its owning queue), and the
    # combine subtracts get any missing >=16 data waits. ---
    id_by_name = {}
    for sid, names in nc.m.ant_sem_names.items():
        for nm in names:
            id_by_name[nm] = int(sid)
    lane_no_by_name = {}
    for nm in id_by_name:
        mm = _re.match(r"DMASW(\d+)_", nm)
        if mm:
            lane_no_by_name[nm] = int(mm.group(1))
    name_by_lane_no = {v: k for k, v in lane_no_by_name.items()}

    # the preps are the only Pool-engine DMAs, so the DMASW lane rotation
    # assigns them lanes 0..3 in scheduled-proc order
    prep_procs = [p.ins.bass_scheduled_proc for p in preps]
    assert len(set(prep_procs)) == 4, prep_procs
    min_proc = min(prep_procs)
    lane_to_gdma = {}
    for i, prep in enumerate(preps):
        lane_no = prep.ins.bass_scheduled_proc - min_proc
        nm = name_by_lane_no[lane_no]
        lane_to_gdma[id_by_name[nm]] = (id_by_name[f"gdma{i}"], f"gdma{i}")

    for blk in nc.m.functions[0].blocks:
        for inst in blk.instructions:
            si = getattr(inst, "sync_info", None)
            if not si or not si.on_wait:
                continue
            for w in si.on_wait:
                if w.id in lane_to_gdma:
                    sid, snm = lane_to_gdma[w.id]
                    w.id = sid
                    w.ant_name = snm

    # ensure each subtract waits on BOTH of its queues' gdma sems
    gdma_ids = {i: id_by_name[f"gdma{i}"] for i in range(4)}
    tmpl = None
    for sub in subs:
        si = sub.ins.sync_info
        if si and si.on_wait:
            tmpl = si.on_wait[0]
            break
    assert tmpl is not None
    for h, sub in enumerate(subs):
        si = sub.ins.sync_info
        have = {w.id for w in (si.on_wait or [])}
        for i in (2 * h, 2 * h + 1):
            sid = gdma_ids[i]
            if sid not in have:
                w = _copy.copy(tmpl)
                w.id = sid
                w.ant_name = f"gdma{i}"
                w.wait_mode = "sem-ge-imm"
                w.wait_value = 16
                si.on_wait.append(w)

    nc.finalize()

    # finalize splits waits into standalone EventSemaphores and can
    # pre-hoist gather-data waits onto non-DVE queues ahead of the cum
    # stores (deadlock); they are redundant there (the DVE combine holds
    # the real data waits; out-store completion covers gather
    # completion), so neutralize them (sem >= 0 is always true).
    gdma_id_set = set(gdma_ids.values())
    for blk in nc.m.functions[0].blocks:
        for inst in blk.instructions:
            if type(inst).__name__ != "InstEventSemaphore":
                continue
            if str(getattr(inst, "engine", None)) == "EngineType.DVE":
                continue
            si = getattr(inst, "sync_info", None)
            if not si or not si.on_wait:
                continue
            for w in si.on_wait:
                if w.id in gdma_id_set:
                    w.wait_value = 0

    return nc


def _make_in_maps(sequence_tensor, span_indices):
    seq = np.asarray(sequence_tensor)
    si32 = np.asarray(span_indices).astype(np.int32)  # values <= 4096: lossless
    assert seq.shape == (B, S, D) and si32.shape == (B, N_SPANS, 2)
    seq16 = np.ascontiguousarray(seq, dtype=np.float16)

    # M[k, p] = 1 iff k>>4 == p>>4 and k < p (block-diag strict upper tri)
    kk = np.arange(P)
    m16 = (((kk[:, None] >> 4) == (kk[None, :] >> 4)) & (kk[:, None] < kk[None, :]))
    m16 = np.ascontiguousarray(m16.astype(np.float16))

    in_maps = []
    for b in range(B):
        sv = si32[b].reshape(P, JG, 2)  # [p, j, (s, e)]
        # [q, c, r] scramble for the Q7 16-partition wrap; 4 blocks of 32
        # cols: [ends j0-3 | ends j4-7 | starts j0-3 | starts j4-7]
        g = sv.reshape(8, 16, JG, 2)  # [r, q, c, k]
        blocks = []
        for k in (1, 0):  # ends first, then starts
            for h in range(2):
                blk = g[:, :, 4 * h : 4 * h + 4, k]  # [r, q, 4]
                blocks.append(blk.transpose(1, 2, 0).reshape(16, 32))
        sp16 = np.concatenate(blocks, axis=1)  # [16, 128]
        sp16 = np.tile(sp16, (8, 1)).astype(np.int16)  # replicate to 128 rows

        s = si32[b, :, 0].astype(np.int64)
        e = si32[b, :, 1].astype(np.int64)
        w = (e - s).astype(np.float32)
        wr = (1.0 / w).reshape(P, JG).astype(np.float32)
        qe = (e - 1) >> 7
        qs = (s - 1) >> 7
        gb = ((qe - qs) * qe).astype(np.int64)  # in [0, 32]
        # ohs[t, j*128 + p] = 1 iff gb[8p + j] == t
        ohs = np.zeros((NT, JG * P), dtype=np.float16)
        n = np.arange(N_SPANS)
        pp, jj = n // JG, n % JG
        ohs[gb, jj * P + pp] = 1.0

        in_maps.append(
            {
                "seq16": seq16[b],
                "spans16": np.ascontiguousarray(sp16),
                "wrec": np.ascontiguousarray(wr),
                "ohs": np.ascontiguousarray(ohs),
                "m16": m16,
            }
        )
    return in_maps


def kernel(sequence_tensor, span_indices):
    from concourse.bass_utils import run_bass_kernel_spmd

    global _cached_nc
    if _cached_nc is None:
        _cached_nc = build_nc()
    in_maps = _make_in_maps(sequence_tensor, span_indices)
    res = run_bass_kernel_spmd(_cached_nc, in_maps, list(range(B)))
    return np.stack([res.results[b]["out"] for b in range(B)], axis=0)


# revision 63
# speedup vs baseline: 1.0325x; 1.0325x over previous
"""Trainium2 Bass kernel for AverageSpanExtractor (segment mean over spans).

Math note: the reference's masked softmax over all-ones logits reduces
exactly to a mean over the span tokens [start, end):
    out[b, n, :] = mean(sequence_tensor[b, start:end, :]).

Strategy (8 cores, batch-parallel - one batch element per core):
  1. Phase 1 builds a DRAM table `cum` (fp16) of block-local inclusive
     prefix sums: cum[1+t] = sum of seq rows [128*(t>>7) .. t].  The
     host ships seq as fp16 (error budget 2e-2; fp16 seq costs ~5e-4)
     and tokens load PACKED: partition p holds tokens 8p..8p+7 of its
     1024-token group, so every load/store descriptor is 4 KB
     contiguous (128/DMA vs 1024 token-major) - HWDGE descriptor rate
     is the phase-1 ceiling otherwise.  The prefix splits into an
     in-place fp16 DVE column chain (7 adds over the 8 packed tokens),
     a host-built block-diagonal strict-triangular matmul M
     (16-partition chunks = 128-token blocks) for the inter-partition
     part, and an fp16 broadcast-add.  The last group's broadcast-add
     and store are split in half to overlap the store with compute.
  2. Span sum = cum[e] - cum[s] + corr; a width<=32 span crosses at
     most one block boundary; corr = Ttab[gb], gb = (qe-qs)*qe,
     q* = (x-1)>>7, Ttab[k] = cum[128k] (row 0 = zeros).
  3. The 2048 cum rows are fetched by four 512-row dma_gather ops on
     four SWDGE queues.  dma_gather lives in the Q7 'mlp' ucode
     library: a dummy 128-row warmup gather right at program start
     absorbs the one-time library load (+drain) that otherwise lands
     between the cum stores and the real gathers.  No other Pool-engine
     library ops exist (M/one-hots/iota work all moved to the host), so
     no mid-program reload.  Desc-gen (~4.5 us Q7 time for the first
     512-row gather, the rest overlap on separate core pairs) starts
     when the last cum store lands.
  4. corr = onehot(gb) @ Ttab on the otherwise-idle PE; one-hots and
     1/width come precomputed from the host (index metadata only).
     Combine = one DVE subtract + batched corr-adds + one batched
     1/w multiply (wrec broadcast along d) per half; output stores
     split across both HWDGE queues.
"""

import numpy as np

B, S, D = 8, 4096, 256
N_SPANS = 1024
P = 128
NBLK = S // P          # 32 blocks of 128 tokens
JG = N_SPANS // P      # 8 spans per partition
MPK = 8                # tokens packed per partition per load group
NG = S // (P * MPK)    # 4 load groups of 1024 tokens
NT = 33                # block-total table rows (incl. zero row)

_cached_nc = None


def build_nc():
    import copy as _copy
    import re as _re

    import concourse.bass as bass
    import concourse.bass_isa as bass_isa
    import concourse.bacc as bacc
    import concourse.mybir as mybir
    from concourse.tile import TileContext
    from concourse.instruction_name_ordered_set import InstructionNameOrderedSet

    f32 = mybir.dt.float32
    f16 = mybir.dt.float16
    i16 = mybir.dt.int16
    Alu = mybir.AluOpType
    Act = mybir.ActivationFunctionType

    nc = bacc.Bacc(
        None, target_bir_lowering=False, debug=False, num_devices=B,
        num_swdge_queues=4,
    )
    seq = nc.declare_dram_parameter("seq16", [S, D], f16, isOutput=False)
    # spans16[q, 32k + c*8 + r] = (k<2 ? end : start) of span
    # 8*(16r+q) + (k%2)*4 + c, replicated across the 8 16-row groups.
    spans16 = nc.declare_dram_parameter("spans16", [P, 128], i16, isOutput=False)
    # host-precomputed metadata: 1/width, corr one-hots, chunk-tri matrix
    wrec = nc.declare_dram_parameter("wrec", [P, JG], f32, isOutput=False)
    ohs = nc.declare_dram_parameter("ohs", [NT, JG * P], f16, isOutput=False)
    m16 = nc.declare_dram_parameter("m16", [P, P], f16, isOutput=False)
    out = nc.declare_dram_parameter("out", [N_SPANS, D], f32, isOutput=True)

    with TileContext(nc) as tc:
        with (
            tc.tile_pool(name="const", bufs=1) as const_pool,
            tc.tile_pool(name="x", bufs=NG) as x_pool,
            tc.tile_pool(name="c", bufs=NG) as c_pool,
            tc.tile_pool(name="ps", bufs=3, space="PSUM") as ps_pool,
            tc.tile_pool(name="cr", bufs=4, space="PSUM") as cr_pool,
            tc.tile_pool(name="misc", bufs=1) as misc_pool,
            tc.tile_pool(name="g", bufs=1) as g_pool,
            tc.tile_pool(name="res", bufs=1) as res_pool,
            tc.tile_pool(name="dram", bufs=1, space="DRAM") as d_pool,
        ):
            # DRAM scratch: block-local prefix rows in fp16; row 0 zeros.
            cum = d_pool.tile([S + 1, D], f16)

            # --- warmup gather: absorbs the Q7 'mlp' library load (+its
            # drain) at program start; result unused ---
            wuidx = misc_pool.tile([P, 8], i16)
            nc.vector.memset(wuidx[:], 0)
            wu = misc_pool.tile([P, D], f16)
            wu_inst = nc.gpsimd.dma_gather(
                wu[:].rearrange("p (c d) -> p c d", d=D),
                seq[:],
                wuidx[:],
                P,
                P,
                D,
                queue_num=0,
            )

            # --- big seq loads, packed [p, m, d]: 4 KB contiguous per
            # partition, 128 descriptors each; alternate SP/ACT queues ---
            bigxs = []
            # high_priority: the scheduler otherwise slots the small
            # metadata loads ahead of these on both HWDGE queues, delaying
            # the first seq descriptors (and everything downstream) ~1.5us
            with tc.high_priority():
                for g in range(NG):
                    t0 = g * MPK * P
                    bigx = x_pool.tile([P, MPK * D], f16, name=f"bigx{g}")
                    eng = nc.sync if g % 2 == 0 else nc.scalar
                    eng.dma_start(
                        out=bigx[:],
                        in_=seq[t0 : t0 + MPK * P, :].rearrange(
                            "(p m) d -> p m d", p=P
                        ),
                    )
                    bigxs.append(bigx)

            # --- small metadata loads ---
            wr = misc_pool.tile([P, JG], f32)
            nc.sync.dma_start(out=wr[:], in_=wrec[:])
            M = const_pool.tile([P, P], f16)
            nc.sync.dma_start(out=M[:], in_=m16[:])
            I16 = misc_pool.tile([P, 128], i16)
            nc.scalar.dma_start(out=I16[:], in_=spans16[:])
            ohS = misc_pool.tile([NT, JG * P], f16)
            nc.scalar.dma_start(out=ohS[:], in_=ohs[:])
            zrow = const_pool.tile([1, D], f16)
            nc.vector.memset(zrow[:], 0.0)
            nc.scalar.dma_start(out=cum[0:1, :], in_=zrow[:])

            # --- gather PREPS: Q7 desc-gen (~4.5 us) reads only I16 and
            # runs on the otherwise-idle Pool engine during phase 1; the
            # cum data read happens at trigger time.  Emitted before the
            # cum stores so no RAW dep lands on the preps.  G is memset
            # first so the scheduling sim (which models prep-written data
            # as ready at prep time) never reads uninitialized memory -
            # the real ordering is enforced by the patched gdma waits. ---
            G = g_pool.tile([P, 16 * D], f16)
            Gv = G[:].rearrange("p (c d) -> p c d", d=D)
            nc.gpsimd.memset(G[:], 0.0)
            preps = []
            for q, k in enumerate((0, 2, 1, 3)):
                dsem = nc.alloc_semaphore(f"gdma{q}")
                preps.append(
                    nc.gpsimd.dma_gather(
                        Gv[:, 4 * k : 4 * (k + 1), :],
                        cum[:],
                        I16[:, 32 * k : 32 * (k + 1)],
                        4 * P,
                        4 * P,
                        D,
                        queue_num=q,
                        prepare_only=True,
                        sem=dsem,
                    )
                )

            # --- phase 1 per group: in-place fp16 column chain
            # (intra-partition prefix over the 8 packed tokens), chunk-tri
            # matmul for the inter-partition part, fp16 broadcast-add,
            # store ---
            bigcs = []
            for g in range(NG):
                t0 = g * MPK * P
                bigx = bigxs[g]
                v = bigx[:].rearrange("p (m d) -> p m d", m=MPK)
                for m in range(1, MPK):
                    nc.vector.tensor_tensor(
                        out=v[:, m, :], in0=v[:, m, :], in1=v[:, m - 1, :],
                        op=Alu.add,
                    )
                part = ps_pool.tile([P, 512], f32)
                nc.tensor.matmul(
                    out=part[:, 0:D], lhsT=M[:], rhs=v[:, MPK - 1, :],
                    start=True, stop=True,
                )
                p16 = misc_pool.tile([P, D], f16, name=f"p16_{g}")
                nc.scalar.activation(out=p16[:], in_=part[:, 0:D], func=Act.Copy)
                bigc = c_pool.tile([P, MPK * D], f16)
                bigcs.append(bigc)
                cv = bigc[:].rearrange("p (m d) -> p m d", m=MPK)
                pb = p16[:].rearrange("p (m d) -> p m d", m=1)
                eng = nc.sync if g % 2 == 0 else nc.scalar
                # split the last group's broadcast-add + store in half so
                # the store overlaps the second half's compute
                halves = (MPK // 2, MPK) if g == NG - 1 else (MPK,)
                mlo = 0
                for mh in halves:
                    nc.vector.tensor_tensor(
                        out=cv[:, mlo:mh, :], in0=v[:, mlo:mh, :],
                        in1=pb.to_broadcast([P, mh - mlo, D]),
                        op=Alu.add,
                    )
                    eng.dma_start(
                        out=cum[1 + t0 : 1 + t0 + MPK * P, :].rearrange(
                            "(p m) d -> p m d", p=P
                        )[:, mlo:mh, :],
                        in_=bigc[:].rearrange("p (m d) -> p m d", m=MPK)[
                            :, mlo:mh, :
                        ],
                    )
                    mlo = mh

            # keep the Q7 cores awake between the warmup and the real
            # gathers: a tiny Pool copy after each group's broadcast-add
            # (first real desc-gen otherwise pays a ~4.5us wake-up)
            for g in range(NG):
                nc.gpsimd.tensor_copy(
                    out=wu[0:1, 2 * g : 2 * g + 2],
                    in_=bigcs[g][0:1, 0:2],
                )

            # --- trigger gating: tiny Pool memsets WAR-depend on each
            # store's source region (and zrow), so they sem-wait store
            # completion ahead of the triggers in the Pool FIFO ---
            gates = [nc.gpsimd.memset(zrow[0:1, 0:4], 0.0)]
            for g in range(NG):
                gates.append(nc.gpsimd.memset(bigcs[g][0:1, 0:4], 0.0))
                if g == NG - 1:
                    gates.append(
                        nc.gpsimd.memset(
                            bigcs[g][0:1, (MPK - 1) * D : (MPK - 1) * D + 4],
                            0.0,
                        )
                    )

            # everything from the triggers on is pinned LAST in the
            # schedule (negative high_priority offset = low priority):
            # the scheduler models prep-written G data as ready at prep
            # time and would otherwise hoist tail ops ahead of phase-1
            # work on shared queues, deadlocking against the
            # store->trigger->gather chain.
            tail = tc.high_priority(offset=-1000000)
            tail.__enter__()
            # manual trigger construction: nosync deps must be attached
            # BEFORE add_instruction for Tile to see them, and the public
            # trigger_dma only links the preps - we also need the gates so
            # neither the scheduler nor the sim fires a trigger before the
            # stores complete.
            trigs = []
            for q in range(4):
                pend = nc.gpsimd._pending_untriggered_insts[q]
                deps = InstructionNameOrderedSet()
                for pi in pend:
                    deps.add(pi.ins.name)
                for gt in gates:
                    deps.add(gt.ins.name)
                kcnt = len(pend)
                nc.gpsimd._pending_untriggered_insts[q] = []
                trigger = bass_isa.InstTriggerDma(
                    name=nc.get_next_instruction_name(),
                    ins=[],
                    outs=[],
                    _count=kcnt,
                    _count_reg=None,
                    queue_num=q,
                )
                trigger.add_nosync_dependencies_from(deps)
                trigs.append(nc.gpsimd.add_instruction(trigger))

            # block-total table: strided fetch of cum rows 0,128,...,4096
            Ttab = misc_pool.tile([NT, D], f16)
            nc.scalar.dma_start(out=Ttab[:], in_=cum[0 : NBLK * P + 1 : P, :])

            # corr_j = onehot(gb_j) @ Ttab on the otherwise-idle PE.
            # Full-bank tiles: a PSUM bank written by PE while DVE reads a
            # bank-mate tile is a fatal HW collision.
            corr = []
            for j in range(JG):
                crj = cr_pool.tile([P, 2 * D], f32)
                nc.tensor.matmul(
                    out=crj[:, 0:D], lhsT=ohS[:, j * P : (j + 1) * P],
                    rhs=Ttab[:], start=True, stop=True,
                )
                corr.append(crj)

            outv = out[:].rearrange("(p jj) d -> p jj d", p=P)
            T1 = res_pool.tile([P, 8 * D], f32)
            T1v = T1[:].rearrange("p (c d) -> p c d", d=D)
            R = res_pool.tile([P, 8 * D], f32)
            Rv = R[:].rearrange("p (c d) -> p c d", d=D)
            subs = []
            for h in range(2):
                # WAW guard: a tiny copy into this sub's output region
                # that RAW-depends on the last broadcast-add, so the
                # scheduler cannot place the sub (which it models as
                # ready at prep time) ahead of phase-1 work on DVE.
                nc.vector.tensor_copy(
                    out=T1[0:1, 4 * h * D : 4 * h * D + 2],
                    in_=bigcs[NG - 1][0:1, 0:2],
                )
                # ends half h lives in Gv cols 4h..4h+4, starts in 8+4h..
                subs.append(
                    nc.vector.tensor_tensor(
                        out=T1v[:, 4 * h : 4 * h + 4, :],
                        in0=Gv[:, 4 * h : 4 * h + 4, :],
                        in1=Gv[:, 8 + 4 * h : 8 + 4 * h + 4, :],
                        op=Alu.subtract,
                    )
                )
                for c in range(4):
                    j = 4 * h + c
                    nc.vector.tensor_tensor(
                        out=T1v[:, j, :], in0=T1v[:, j, :], in1=corr[j][:, 0:D],
                        op=Alu.add,
                    )
                    nc.scalar.activation(
                        out=Rv[:, j, :], in_=T1v[:, j, :], func=Act.Copy,
                        scale=wr[:, j : j + 1],
                    )
                eng = nc.sync if h == 0 else nc.scalar
                eng.dma_start(
                    out=outv[:, 4 * h : 4 * h + 4, :],
                    in_=Rv[:, 4 * h : 4 * h + 4, :],
                )
            tail.__exit__(None, None, None)

    # --- post-pass surgery (Tile's managed path does not support
    # DRAM-source preps): every WAIT referencing a prep's Tile-assigned
    # DMASW lane sem is remapped to the prep's descriptor-encoded gdma
    # sem (a lane sem may only be updated by its owning queue), and the
    # combine subtracts get any missing >=16 data waits. ---
    id_by_name = {}
    for sid, names in nc.m.ant_sem_names.items():
        for nm in names:
            id_by_name[nm] = int(sid)
    lane_no_by_name = {}
    for nm in id_by_name:
        mm = _re.match(r"DMASW(\d+)_", nm)
        if mm:
            lane_no_by_name[nm] = int(mm.group(1))
    name_by_lane_no = {v: k for k, v in lane_no_by_name.items()}

    wu_proc = wu_inst.ins.bass_scheduled_proc
    wu_lane = None
    for u in wu_inst.ins.sync_info.on_update or []:
        if u.ant_name in lane_no_by_name:
            wu_lane = lane_no_by_name[u.ant_name]
    assert wu_lane is not None
    prep_procs = [p.ins.bass_scheduled_proc for p in preps]
    assert len(set(prep_procs)) == 4, prep_procs
    lane_to_gdma = {}
    for i, prep in enumerate(preps):
        lane_no = wu_lane + (prep.ins.bass_scheduled_proc - wu_proc)
        nm = name_by_lane_no[lane_no]
        lane_to_gdma[id_by_name[nm]] = (id_by_name[f"gdma{i}"], f"gdma{i}")

    for blk in nc.m.functions[0].blocks:
        for inst in blk.instructions:
            si = getattr(inst, "sync_info", None)
            if not si or not si.on_wait:
                continue
            for w in si.on_wait:
                if w.id in lane_to_gdma:
                    sid, snm = lane_to_gdma[w.id]
                    w.id = sid
                    w.ant_name = snm

    # ensure each subtract waits on BOTH of its queues' gdma sems
    gdma_ids = {i: id_by_name[f"gdma{i}"] for i in range(4)}
    tmpl = None
    for sub in subs:
        si = sub.ins.sync_info
        if si and si.on_wait:
            tmpl = si.on_wait[0]
            break
    assert tmpl is not None
    for h, sub in enumerate(subs):
        si = sub.ins.sync_info
        have = {w.id for w in (si.on_wait or [])}
        for i in (2 * h, 2 * h + 1):
            sid = gdma_ids[i]
            if sid not in have:
                w = _copy.copy(tmpl)
                w.id = sid
                w.ant_name = f"gdma{i}"
                w.wait_mode = "sem-ge-imm"
                w.wait_value = 16
                si.on_wait.append(w)

    nc.finalize()

    # finalize splits waits into standalone EventSemaphores and can
    # pre-hoist gather-data waits onto non-DVE queues ahead of the cum
    # stores (deadlock); they are redundant there (the DVE combine holds
    # the real data waits; out-store completion covers gather
    # completion), so neutralize them (sem >= 0 is always true).
    gdma_id_set = set(gdma_ids.values())
    for blk in nc.m.functions[0].blocks:
        for inst in blk.instructions:
            if type(inst).__name__ != "InstEventSemaphore":
                continue
            if str(getattr(inst, "engine", None)) == "EngineType.DVE":
                continue
            si = getattr(inst, "sync_info", None)
            if not si or not si.on_wait:
                continue
            for w in si.on_wait:
                if w.id in gdma_id_set:
                    w.wait_value = 0

    return nc


def _make_in_maps(sequence_tensor, span_indices):
    seq = np.asarray(sequence_tensor)
    si32 = np.asarray(span_indices).astype(np.int32)  # values <= 4096: lossless
    assert seq.shape == (B, S, D) and si32.shape == (B, N_SPANS, 2)
    seq16 = np.ascontiguousarray(seq, dtype=np.float16)

    # M[k, p] = 1 iff k>>4 == p>>4 and k < p (block-diag strict upper tri)
    kk = np.arange(P)
    m16 = (((kk[:, None] >> 4) == (kk[None, :] >> 4)) & (kk[:, None] < kk[None, :]))
    m16 = np.ascontiguousarray(m16.astype(np.float16))

    in_maps = []
    for b in range(B):
        sv = si32[b].reshape(P, JG, 2)  # [p, j, (s, e)]
        # [q, c, r] scramble for the Q7 16-partition wrap; 4 blocks of 32
        # cols: [ends j0-3 | ends j4-7 | starts j0-3 | starts j4-7]
        g = sv.reshape(8, 16, JG, 2)  # [r, q, c, k]
        blocks = []
        for k in (1, 0):  # ends first, then starts
            for h in range(2):
                blk = g[:, :, 4 * h : 4 * h + 4, k]  # [r, q, 4]
                blocks.append(blk.transpose(1, 2, 0).reshape(16, 32))
        sp16 = np.concatenate(blocks, axis=1)  # [16, 128]
        sp16 = np.tile(sp16, (8, 1)).astype(np.int16)  # replicate to 128 rows

        s = si32[b, :, 0].astype(np.int64)
        e = si32[b, :, 1].astype(np.int64)
        w = (e - s).astype(np.float32)
        wr = (1.0 / w).reshape(P, JG).astype(np.float32)
        qe = (e - 1) >> 7
        qs = (s - 1) >> 7
        gb = ((qe - qs) * qe).astype(np.int64)  # in [0, 32]
        # ohs[t, j*128 + p] = 1 iff gb[8p + j] == t
        ohs = np.zeros((NT, JG * P), dtype=np.float16)
        n = np.arange(N_SPANS)
        pp, jj = n // JG, n % JG
        ohs[gb, jj * P + pp] = 1.0

        in_maps.append(
            {
                "seq16": seq16[b],
                "spans16": np.ascontiguousarray(sp16),
                "wrec": np.ascontiguousarray(wr),
                "ohs": np.ascontiguousarray(ohs),
                "m16": m16,
            }
        )
    return in_maps


def kernel(sequence_tensor, span_indices):
    from concourse.bass_utils import run_bass_kernel_spmd

    global _cached_nc
    if _cached_nc is None:
        _cached_nc = build_nc()
    in_maps = _make_in_maps(sequence_tensor, span_indices)
    res = run_bass_kernel_spmd(_cached_nc, in_maps, list(range(B)))
    return np.stack([res.results[b]["out"] for b in range(B)], axis=0)


# revision 64
# speedup vs baseline: 1.0614x; 1.0280x over previous
"""Trainium2 Bass kernel for AverageSpanExtractor (segment mean over spans).

Math note: the reference's masked softmax over all-ones logits reduces
exactly to a mean over the span tokens [start, end):
    out[b, n, :] = mean(sequence_tensor[b, start:end, :]).

Strategy (8 cores, batch-parallel - one batch element per core):
  1. Phase 1 builds a DRAM table `cum` (fp16) of block-local inclusive
     prefix sums: cum[1+t] = sum of seq rows [128*(t>>7) .. t].  The
     host ships seq as fp16 (error budget 2e-2; fp16 seq costs ~5e-4)
     and tokens load PACKED: partition p holds tokens 8p..8p+7 of its
     1024-token group, so every load/store descriptor is 4 KB
     contiguous (128/DMA vs 1024 token-major) - HWDGE descriptor rate
     is the phase-1 ceiling otherwise.  The prefix splits into an
     in-place fp16 DVE column chain (7 adds over the 8 packed tokens),
     a host-built block-diagonal strict-triangular matmul M
     (16-partition chunks = 128-token blocks) for the inter-partition
     part, and an fp16 broadcast-add.  The last group's broadcast-add
     and store are split in half to overlap the store with compute.
  2. Span sum = cum[e] - cum[s] + corr; a width<=32 span crosses at
     most one block boundary; corr = Ttab[gb], gb = (qe-qs)*qe,
     q* = (x-1)>>7, Ttab[k] = cum[128k] (row 0 = zeros).
  3. The 2048 cum rows are fetched by four 512-row dma_gather ops on
     four SWDGE queues.  dma_gather lives in the Q7 'mlp' ucode
     library: a dummy 128-row warmup gather right at program start
     absorbs the one-time library load (+drain) that otherwise lands
     between the cum stores and the real gathers.  No other Pool-engine
     library ops exist (M/one-hots/iota work all moved to the host), so
     no mid-program reload.  Desc-gen (~4.5 us Q7 time for the first
     512-row gather, the rest overlap on separate core pairs) starts
     when the last cum store lands.
  4. corr = onehot(gb) @ Ttab on the otherwise-idle PE; one-hots and
     1/width come precomputed from the host (index metadata only).
     Combine = one DVE subtract + batched corr-adds + one batched
     1/w multiply (wrec broadcast along d) per half; output stores
     split across both HWDGE queues.
"""

import numpy as np

B, S, D = 8, 4096, 256
N_SPANS = 1024
P = 128
NBLK = S // P          # 32 blocks of 128 tokens
JG = N_SPANS // P      # 8 spans per partition
MPK = 8                # tokens packed per partition per load group
NG = S // (P * MPK)    # 4 load groups of 1024 tokens
NT = 33                # block-total table rows (incl. zero row)

_cached_nc = None


def build_nc():
    import copy as _copy
    import re as _re

    import concourse.bass as bass
    import concourse.bass_isa as bass_isa
    import concourse.bacc as bacc
    import concourse.mybir as mybir
    from concourse.tile import TileContext
    from concourse.instruction_name_ordered_set import InstructionNameOrderedSet

    f32 = mybir.dt.float32
    f16 = mybir.dt.float16
    i16 = mybir.dt.int16
    Alu = mybir.AluOpType
    Act = mybir.ActivationFunctionType

    nc = bacc.Bacc(
        None, target_bir_lowering=False, debug=False, num_devices=B,
        num_swdge_queues=4,
    )
    seq = nc.declare_dram_parameter("seq16", [S, D], f16, isOutput=False)
    # spans16[q, 32k + c*8 + r] = (k<2 ? end : start) of span
    # 8*(16r+q) + (k%2)*4 + c, replicated across the 8 16-row groups.
    spans16 = nc.declare_dram_parameter("spans16", [P, 128], i16, isOutput=False)
    # host-precomputed metadata: 1/width, corr one-hots, chunk-tri matrix
    wrec = nc.declare_dram_parameter("wrec", [P, JG], f32, isOutput=False)
    ohs = nc.declare_dram_parameter("ohs", [NT, JG * P], f16, isOutput=False)
    m16 = nc.declare_dram_parameter("m16", [P, P], f16, isOutput=False)
    out = nc.declare_dram_parameter("out", [N_SPANS, D], f16, isOutput=True)

    with TileContext(nc) as tc:
        with (
            tc.tile_pool(name="const", bufs=1) as const_pool,
            tc.tile_pool(name="x", bufs=NG) as x_pool,
            tc.tile_pool(name="c", bufs=NG) as c_pool,
            tc.tile_pool(name="ps", bufs=3, space="PSUM") as ps_pool,
            tc.tile_pool(name="cr", bufs=4, space="PSUM") as cr_pool,
            tc.tile_pool(name="misc", bufs=1) as misc_pool,
            tc.tile_pool(name="g", bufs=1) as g_pool,
            tc.tile_pool(name="res", bufs=1) as res_pool,
            tc.tile_pool(name="dram", bufs=1, space="DRAM") as d_pool,
        ):
            # DRAM scratch: block-local prefix rows in fp16; row 0 zeros.
            cum = d_pool.tile([S + 1, D], f16)

            # --- warmup gather: absorbs the Q7 'mlp' library load (+its
            # drain) at program start; result unused ---
            wuidx = misc_pool.tile([P, 8], i16)
            nc.vector.memset(wuidx[:], 0)
            wu = misc_pool.tile([P, D], f16)
            wu_inst = nc.gpsimd.dma_gather(
                wu[:].rearrange("p (c d) -> p c d", d=D),
                seq[:],
                wuidx[:],
                P,
                P,
                D,
                queue_num=0,
            )

            # --- big seq loads, packed [p, m, d]: 4 KB contiguous per
            # partition, 128 descriptors each; alternate SP/ACT queues ---
            bigxs = []
            # high_priority: the scheduler otherwise slots the small
            # metadata loads ahead of these on both HWDGE queues, delaying
            # the first seq descriptors (and everything downstream) ~1.5us
            with tc.high_priority():
                for g in range(NG):
                    t0 = g * MPK * P
                    bigx = x_pool.tile([P, MPK * D], f16, name=f"bigx{g}")
                    eng = nc.sync if g % 2 == 0 else nc.scalar
                    eng.dma_start(
                        out=bigx[:],
                        in_=seq[t0 : t0 + MPK * P, :].rearrange(
                            "(p m) d -> p m d", p=P
                        ),
                    )
                    bigxs.append(bigx)

            # --- small metadata loads ---
            wr = misc_pool.tile([P, JG], f32)
            nc.sync.dma_start(out=wr[:], in_=wrec[:])
            M = const_pool.tile([P, P], f16)
            nc.sync.dma_start(out=M[:], in_=m16[:])
            I16 = misc_pool.tile([P, 128], i16)
            nc.scalar.dma_start(out=I16[:], in_=spans16[:])
            ohS = misc_pool.tile([NT, JG * P], f16)
            nc.scalar.dma_start(out=ohS[:], in_=ohs[:])
            zrow = const_pool.tile([1, D], f16)
            nc.vector.memset(zrow[:], 0.0)
            nc.scalar.dma_start(out=cum[0:1, :], in_=zrow[:])

            # --- gather PREPS: Q7 desc-gen (~4.5 us) reads only I16 and
            # runs on the otherwise-idle Pool engine during phase 1; the
            # cum data read happens at trigger time.  Emitted before the
            # cum stores so no RAW dep lands on the preps.  G is memset
            # first so the scheduling sim (which models prep-written data
            # as ready at prep time) never reads uninitialized memory -
            # the real ordering is enforced by the patched gdma waits. ---
            G = g_pool.tile([P, 16 * D], f16)
            Gv = G[:].rearrange("p (c d) -> p c d", d=D)
            nc.gpsimd.memset(G[:], 0.0)
            preps = []
            for q, k in enumerate((0, 2, 1, 3)):
                dsem = nc.alloc_semaphore(f"gdma{q}")
                preps.append(
                    nc.gpsimd.dma_gather(
                        Gv[:, 4 * k : 4 * (k + 1), :],
                        cum[:],
                        I16[:, 32 * k : 32 * (k + 1)],
                        4 * P,
                        4 * P,
                        D,
                        queue_num=q,
                        prepare_only=True,
                        sem=dsem,
                    )
                )

            # --- phase 1 per group: in-place fp16 column chain
            # (intra-partition prefix over the 8 packed tokens), chunk-tri
            # matmul for the inter-partition part, fp16 broadcast-add,
            # store ---
            bigcs = []
            for g in range(NG):
                t0 = g * MPK * P
                bigx = bigxs[g]
                v = bigx[:].rearrange("p (m d) -> p m d", m=MPK)
                for m in range(1, MPK):
                    nc.vector.tensor_tensor(
                        out=v[:, m, :], in0=v[:, m, :], in1=v[:, m - 1, :],
                        op=Alu.add,
                    )
                part = ps_pool.tile([P, 512], f32)
                nc.tensor.matmul(
                    out=part[:, 0:D], lhsT=M[:], rhs=v[:, MPK - 1, :],
                    start=True, stop=True,
                )
                p16 = misc_pool.tile([P, D], f16, name=f"p16_{g}")
                nc.scalar.activation(out=p16[:], in_=part[:, 0:D], func=Act.Copy)
                bigc = c_pool.tile([P, MPK * D], f16)
                bigcs.append(bigc)
                cv = bigc[:].rearrange("p (m d) -> p m d", m=MPK)
                pb = p16[:].rearrange("p (m d) -> p m d", m=1)
                eng = nc.sync if g % 2 == 0 else nc.scalar
                # split the last group's broadcast-add + store in half so
                # the store overlaps the second half's compute
                halves = (MPK // 2, MPK) if g == NG - 1 else (MPK,)
                mlo = 0
                for mh in halves:
                    nc.vector.tensor_tensor(
                        out=cv[:, mlo:mh, :], in0=v[:, mlo:mh, :],
                        in1=pb.to_broadcast([P, mh - mlo, D]),
                        op=Alu.add,
                    )
                    eng.dma_start(
                        out=cum[1 + t0 : 1 + t0 + MPK * P, :].rearrange(
                            "(p m) d -> p m d", p=P
                        )[:, mlo:mh, :],
                        in_=bigc[:].rearrange("p (m d) -> p m d", m=MPK)[
                            :, mlo:mh, :
                        ],
                    )
                    mlo = mh

            # keep the Q7 cores awake between the warmup and the real
            # gathers: a tiny Pool copy after each group's broadcast-add
            # (first real desc-gen otherwise pays a ~4.5us wake-up)
            for g in range(NG):
                nc.gpsimd.tensor_copy(
                    out=wu[0:1, 2 * g : 2 * g + 2],
                    in_=bigcs[g][0:1, 0:2],
                )

            # --- trigger gating: tiny Pool memsets WAR-depend on each
            # store's source region (and zrow), so they sem-wait store
            # completion ahead of the triggers in the Pool FIFO ---
            gates = [nc.gpsimd.memset(zrow[0:1, 0:4], 0.0)]
            for g in range(NG):
                gates.append(nc.gpsimd.memset(bigcs[g][0:1, 0:4], 0.0))
                if g == NG - 1:
                    gates.append(
                        nc.gpsimd.memset(
                            bigcs[g][0:1, (MPK - 1) * D : (MPK - 1) * D + 4],
                            0.0,
                        )
                    )

            # everything from the triggers on is pinned LAST in the
            # schedule (negative high_priority offset = low priority):
            # the scheduler models prep-written G data as ready at prep
            # time and would otherwise hoist tail ops ahead of phase-1
            # work on shared queues, deadlocking against the
            # store->trigger->gather chain.
            tail = tc.high_priority(offset=-1000000)
            tail.__enter__()
            # manual trigger construction: nosync deps must be attached
            # BEFORE add_instruction for Tile to see them, and the public
            # trigger_dma only links the preps - we also need the gates so
            # neither the scheduler nor the sim fires a trigger before the
            # stores complete.
            trigs = []
            for q in range(4):
                pend = nc.gpsimd._pending_untriggered_insts[q]
                deps = InstructionNameOrderedSet()
                for pi in pend:
                    deps.add(pi.ins.name)
                for gt in gates:
                    deps.add(gt.ins.name)
                kcnt = len(pend)
                nc.gpsimd._pending_untriggered_insts[q] = []
                trigger = bass_isa.InstTriggerDma(
                    name=nc.get_next_instruction_name(),
                    ins=[],
                    outs=[],
                    _count=kcnt,
                    _count_reg=None,
                    queue_num=q,
                )
                trigger.add_nosync_dependencies_from(deps)
                trigs.append(nc.gpsimd.add_instruction(trigger))

            # block-total table: strided fetch of cum rows 0,128,...,4096
            Ttab = misc_pool.tile([NT, D], f16)
            nc.scalar.dma_start(out=Ttab[:], in_=cum[0 : NBLK * P + 1 : P, :])

            # corr_j = onehot(gb_j) @ Ttab on the otherwise-idle PE.
            # Full-bank tiles: a PSUM bank written by PE while DVE reads a
            # bank-mate tile is a fatal HW collision.
            corr = []
            for j in range(JG):
                crj = cr_pool.tile([P, 2 * D], f32)
                nc.tensor.matmul(
                    out=crj[:, 0:D], lhsT=ohS[:, j * P : (j + 1) * P],
                    rhs=Ttab[:], start=True, stop=True,
                )
                corr.append(crj)

            outv = out[:].rearrange("(p jj) d -> p jj d", p=P)
            T1 = res_pool.tile([P, 8 * D], f32)
            T1v = T1[:].rearrange("p (c d) -> p c d", d=D)
            R = res_pool.tile([P, 8 * D], f16)
            Rv = R[:].rearrange("p (c d) -> p c d", d=D)
            subs = []
            for h in range(2):
                # WAW guard: a tiny copy into this sub's output region
                # that RAW-depends on the last broadcast-add, so the
                # scheduler cannot place the sub (which it models as
                # ready at prep time) ahead of phase-1 work on DVE.
                nc.vector.tensor_copy(
                    out=T1[0:1, 4 * h * D : 4 * h * D + 2],
                    in_=bigcs[NG - 1][0:1, 0:2],
                )
                # ends half h lives in Gv cols 4h..4h+4, starts in 8+4h..
                subs.append(
                    nc.vector.tensor_tensor(
                        out=T1v[:, 4 * h : 4 * h + 4, :],
                        in0=Gv[:, 4 * h : 4 * h + 4, :],
                        in1=Gv[:, 8 + 4 * h : 8 + 4 * h + 4, :],
                        op=Alu.subtract,
                    )
                )
                for c in range(4):
                    j = 4 * h + c
                    nc.vector.tensor_tensor(
                        out=T1v[:, j, :], in0=T1v[:, j, :], in1=corr[j][:, 0:D],
                        op=Alu.add,
                    )
                    nc.scalar.activation(
                        out=Rv[:, j, :], in_=T1v[:, j, :], func=Act.Copy,
                        scale=wr[:, j : j + 1],
                    )
                eng = nc.sync if h == 0 else nc.scalar
                eng.dma_start(
                    out=outv[:, 4 * h : 4 * h + 4, :],
                    in_=Rv[:, 4 * h : 4 * h + 4, :],
                )
            tail.__exit__(None, None, None)

    # --- post-pass surgery (Tile's managed path does not support
    # DRAM-source preps): every WAIT referencing a prep's Tile-assigned
    # DMASW lane sem is remapped to the prep's descriptor-encoded gdma
    # sem (a lane sem may only be updated by its owning queue), and the
    # combine subtracts get any missing >=16 data waits. ---
    id_by_name = {}
    for sid, names in nc.m.ant_sem_names.items():
        for nm in names:
            id_by_name[nm] = int(sid)
    lane_no_by_name = {}
    for nm in id_by_name:
        mm = _re.match(r"DMASW(\d+)_", nm)
        if mm:
            lane_no_by_name[nm] = int(mm.group(1))
    name_by_lane_no = {v: k for k, v in lane_no_by_name.items()}

    wu_proc = wu_inst.ins.bass_scheduled_proc
    wu_lane = None
    for u in wu_inst.ins.sync_info.on_update or []:
        if u.ant_name in lane_no_by_name:
            wu_lane = lane_no_by_name[u.ant_name]
    assert wu_lane is not None
    prep_procs = [p.ins.bass_scheduled_proc for p in preps]
    assert len(set(prep_procs)) == 4, prep_procs
    lane_to_gdma = {}
    for i, prep in enumerate(preps):
        lane_no = wu_lane + (prep.ins.bass_scheduled_proc - wu_proc)
        nm = name_by_lane_no[lane_no]
        lane_to_gdma[id_by_name[nm]] = (id_by_name[f"gdma{i}"], f"gdma{i}")

    for blk in nc.m.functions[0].blocks:
        for inst in blk.instructions:
            si = getattr(inst, "sync_info", None)
            if not si or not si.on_wait:
                continue
            for w in si.on_wait:
                if w.id in lane_to_gdma:
                    sid, snm = lane_to_gdma[w.id]
                    w.id = sid
                    w.ant_name = snm

    # ensure each subtract waits on BOTH of its queues' gdma sems
    gdma_ids = {i: id_by_name[f"gdma{i}"] for i in range(4)}
    tmpl = None
    for sub in subs:
        si = sub.ins.sync_info
        if si and si.on_wait:
            tmpl = si.on_wait[0]
            break
    assert tmpl is not None
    for h, sub in enumerate(subs):
        si = sub.ins.sync_info
        have = {w.id for w in (si.on_wait or [])}
        for i in (2 * h, 2 * h + 1):
            sid = gdma_ids[i]
            if sid not in have:
                w = _copy.copy(tmpl)
                w.id = sid
                w.ant_name = f"gdma{i}"
                w.wait_mode = "sem-ge-imm"
                w.wait_value = 16
                si.on_wait.append(w)

    nc.finalize()

    # finalize splits waits into standalone EventSemaphores and can
    # pre-hoist gather-data waits onto non-DVE queues ahead of the cum
    # stores (deadlock); they are redundant there (the DVE combine holds
    # the real data waits; out-store completion covers gather
    # completion), so neutralize them (sem >= 0 is always true).
    gdma_id_set = set(gdma_ids.values())
    for blk in nc.m.functions[0].blocks:
        for inst in blk.instructions:
            if type(inst).__name__ != "InstEventSemaphore":
                continue
            if str(getattr(inst, "engine", None)) == "EngineType.DVE":
                continue
            si = getattr(inst, "sync_info", None)
            if not si or not si.on_wait:
                continue
            for w in si.on_wait:
                if w.id in gdma_id_set:
                    w.wait_value = 0

    return nc


def _make_in_maps(sequence_tensor, span_indices):
    seq = np.asarray(sequence_tensor)
    si32 = np.asarray(span_indices).astype(np.int32)  # values <= 4096: lossless
    assert seq.shape == (B, S, D) and si32.shape == (B, N_SPANS, 2)
    seq16 = np.ascontiguousarray(seq, dtype=np.float16)

    # M[k, p] = 1 iff k>>4 == p>>4 and k < p (block-diag strict upper tri)
    kk = np.arange(P)
    m16 = (((kk[:, None] >> 4) == (kk[None, :] >> 4)) & (kk[:, None] < kk[None, :]))
    m16 = np.ascontiguousarray(m16.astype(np.float16))

    in_maps = []
    for b in range(B):
        sv = si32[b].reshape(P, JG, 2)  # [p, j, (s, e)]
        # [q, c, r] scramble for the Q7 16-partition wrap; 4 blocks of 32
        # cols: [ends j0-3 | ends j4-7 | starts j0-3 | starts j4-7]
        g = sv.reshape(8, 16, JG, 2)  # [r, q, c, k]
        blocks = []
        for k in (1, 0):  # ends first, then starts
            for h in range(2):
                blk = g[:, :, 4 * h : 4 * h + 4, k]  # [r, q, 4]
                blocks.append(blk.transpose(1, 2, 0).reshape(16, 32))
        sp16 = np.concatenate(blocks, axis=1)  # [16, 128]
        sp16 = np.tile(sp16, (8, 1)).astype(np.int16)  # replicate to 128 rows

        s = si32[b, :, 0].astype(np.int64)
        e = si32[b, :, 1].astype(np.int64)
        w = (e - s).astype(np.float32)
        wr = (1.0 / w).reshape(P, JG).astype(np.float32)
        qe = (e - 1) >> 7
        qs = (s - 1) >> 7
        gb = ((qe - qs) * qe).astype(np.int64)  # in [0, 32]
        # ohs[t, j*128 + p] = 1 iff gb[8p + j] == t
        ohs = np.zeros((NT, JG * P), dtype=np.float16)
        n = np.arange(N_SPANS)
        pp, jj = n // JG, n % JG
        ohs[gb, jj * P + pp] = 1.0

        in_maps.append(
            {
                "seq16": seq16[b],
                "spans16": np.ascontiguousarray(sp16),
                "wrec": np.ascontiguousarray(wr),
                "ohs": np.ascontiguousarray(ohs),
                "m16": m16,
            }
        )
    return in_maps


def kernel(sequence_tensor, span_indices):
    from concourse.bass_utils import run_bass_kernel_spmd

    global _cached_nc
    if _cached_nc is None:
        _cached_nc = build_nc()
    in_maps = _make_in_maps(sequence_tensor, span_indices)
    res = run_bass_kernel_spmd(_cached_nc, in_maps, list(range(B)))
    return np.stack(
        [res.results[b]["out"] for b in range(B)], axis=0
    ).astype(np.float32)
